# revision 15
# baseline (speedup 1.0000x reference)
"""Sparse Adagrad (Habana-style) on 8 Trainium2 NeuronCores.

Strategy: row-shard the embedding tables (weights/moments) across the 8
cores by index range (62500 rows each, padded to 63488 = 128*496). The
host routes each valid gradient row to its owning core. On device, each
core sweeps its table shard once with large contiguous DMAs; the sparse
scatter-add (with duplicate indices) is done with one-hot matmuls on the
TensorEngine accumulating into PSUM, so duplicates sum natively.

Table layout per core: row r -> SBUF partition p = r // 496, free offset
j = r % 496 (so a [63488, 64] shard is exactly a [128, 496*64] SBUF
sweep with contiguous per-partition DMA). Tables travel in fp16 (host
converts f32<->fp16); with |w|<6, m<40 the fp16 relative error ~5e-4 is
far inside the tolerance, and it halves the HBM traffic of the sweep.

Per block j (the 128 rows {p*496 + j}), the host packs the gradient rows
whose local index maps to block j into up to CPB chunks of `cap` "slots"
(slot -> partition). A one-hot matrix A[slot, p] = (strip(slot) == p)
is built on device (tensor_scalar is_equal against an iota, per-partition
scalar = the slot's strip index), and
    psum_m[p, :] += A.T @ g2_chunk      (moment increments, Sum g^2)
    psum_g[p, :] += A.T @ g_chunk       (gradient sums, Sum g)
Then the update (denominator uses the fully accumulated moment, and it
is constant across duplicates so it factors out of the sum):
    m' = m + psum_m
    w' = w - lr * psum_g * rsqrt(m' + 1e-20)
"""

import sys

for _p in ("/opt/trn_rl_repo", "/root/.axon_site/_ro/trn_rl_repo"):
    if _p not in sys.path:
        sys.path.insert(0, _p)

import numpy as np

P = 128          # SBUF partitions / matmul output rows
D = 64           # embedding dim
NCORES = 8
VC = 62500       # table rows per core
R = 496          # rows per strip (= blocks per core); 128*496 = 63488 >= VC
PADV = P * R     # padded rows per core
JSUB = 16        # blocks per sweep iteration (PSUM limited)
NIT = R // JSUB  # 31 sweep iterations

_program_cache = {}


def _build_program(cpb, cap, jsub=JSUB, sbufs=3, pbufs=2, store_engine="scalar",
                   g_dtype="fp16", t_dtype="fp16", g_load_engine="sync",
                   onehot="ts", minject=True, nb=2, reps=1, loop_reps=False):
    from concourse import bacc, mybir
    import concourse.tile as tile

    nit = R // jsub
    assert nit * jsub == R
    f32 = mybir.dt.float32
    nc = bacc.Bacc("TRN2", target_bir_lowering=False, debug=False,
                   num_devices=NCORES)

    dtmap = {"f32": f32, "bf16": mybir.dt.bfloat16, "fp16": mybir.dt.float16}
    gdt = dtmap[g_dtype]
    tdt = dtmap[t_dtype]

    w_in = nc.dram_tensor("w_in", [P, R * D], tdt, kind="ExternalInput")
    m_in = nc.dram_tensor("m_in", [P, R * D], tdt, kind="ExternalInput")
    g_in = nc.dram_tensor("g_in", [cap, R * cpb * D], gdt,
                          kind="ExternalInput")
    midx = nc.dram_tensor("midx", [cap, R * cpb], f32, kind="ExternalInput")
    lr_in = nc.dram_tensor("lr", [1, 1], f32, kind="ExternalInput")
    w_out = nc.dram_tensor("w_out", [P, R * D], tdt, kind="ExternalOutput")
    m_out = nc.dram_tensor("m_out", [P, R * D], tdt, kind="ExternalOutput")

    with tile.TileContext(nc) as tc:
        with tc.tile_pool(name="consts", bufs=1) as consts, \
             tc.tile_pool(name="sbuf", bufs=sbufs) as pool, \
             tc.tile_pool(name="psum", bufs=pbufs, space="PSUM") as psum:
            iota_i = consts.tile([P, P], mybir.dt.int32)
            nc.gpsimd.iota(iota_i[:], pattern=[[1, P]], base=0,
                           channel_multiplier=0)
            iota_g = consts.tile([P, P], gdt)
            nc.vector.tensor_copy(iota_g[:], iota_i[:])

            # per-partition index p as f32 [P, 1] (tensor_scalar scalar slot)
            iotap_i = consts.tile([P, 1], mybir.dt.int32)
            nc.gpsimd.iota(iotap_i[:], pattern=[[1, 1]], base=0,
                           channel_multiplier=1)
            iotap_f = consts.tile([P, 1], f32)
            nc.vector.tensor_copy(iotap_f[:], iotap_i[:])

            # identity matrix in gdt, for injecting m into psum_m via PE
            ident = consts.tile([P, P], gdt)
            nc.vector.tensor_scalar(out=ident[:], in0=iota_g[:],
                                    scalar1=iotap_f[:], scalar2=None,
                                    op0=mybir.AluOpType.is_equal)

            eps_t = consts.tile([P, 1], f32)
            nc.gpsimd.memset(eps_t[:], 1e-20)

            # inv_s2 = 1 / stream_scale^2 (precomputed on host), used to
            # recover Sum g^2 from Sum (stream_scale*g)^2
            inv_s2 = consts.tile([P, 1], f32)
            nc.sync.dma_start(out=inv_s2[:], in_=lr_in[:].to_broadcast((P, 1)))

            midx_s = consts.tile([cap, R * cpb], f32)
            nc.sync.dma_start(out=midx_s[:], in_=midx[:])

            store = getattr(nc, store_engine)

            import contextlib

            def _rep_scope():
                if loop_reps and reps > 1:
                    return tc.For_i(0, reps, 1)
                return contextlib.nullcontext()

            JD = jsub * D
            KD = jsub * cpb * D
            KK = jsub * cpb

            with _rep_scope():
              for _rep in range(1 if loop_reps else reps):
                for grp in range(-(-nit // nb)):
                    # group-level DMAs: one load/store per nbe iterations
                    nbe = min(nb, nit - grp * nb)
                    gc0 = grp * nb * JD
                    gs0 = grp * nb * KD

                    w_s = pool.tile([P, nbe * JD], tdt)
                    nc.sync.dma_start(out=w_s[:],
                                      in_=w_in[:, gc0:gc0 + nbe * JD])
                    m_s = pool.tile([P, nbe * JD], tdt)
                    nc.sync.dma_start(out=m_s[:],
                                      in_=m_in[:, gc0:gc0 + nbe * JD])
                    g_s = pool.tile([cap, nbe * KD], gdt)
                    getattr(nc, g_load_engine).dma_start(
                        out=g_s[:], in_=g_in[:, gs0:gs0 + nbe * KD])

                    g2_s = pool.tile([cap, nbe * KD], gdt)
                    nc.scalar.square(g2_s[:], g_s[:])

                    m_n = pool.tile([P, nbe * JD], tdt)
                    w_n = pool.tile([P, nbe * JD], tdt)

                    for i2 in range(nbe):
                        it = grp * nb + i2
                        k0 = it * KK
                        o0 = i2 * JD          # offset in group w/m tiles
                        q0 = i2 * KD          # offset in group g tiles

                        a_s = pool.tile([cap, KK, P], gdt)
                        if onehot == "ts":
                            for k in range(KK):
                                nc.vector.tensor_scalar(
                                    out=a_s[:, k, :],
                                    in0=iota_g[:cap, :],
                                    scalar1=midx_s[:, k0 + k:k0 + k + 1],
                                    scalar2=None,
                                    op0=mybir.AluOpType.is_equal,
                                )
                        else:
                            nc.vector.tensor_tensor(
                                out=a_s[:],
                                in0=midx_s[:, k0:k0 + KK, None]
                                    .broadcast_to((cap, KK, P)),
                                in1=iota_g[:cap, None, :].broadcast_to(
                                    (cap, KK, P)),
                                op=mybir.AluOpType.is_equal,
                            )

                        psum_m = psum.tile([P, JD], f32)
                        psum_g = psum.tile([P, JD], f32)
                        # all psum_m matmuls first so the m-chain (sqrt ->
                        # recip) overlaps with the psum_g matmuls
                        for jj in range(jsub):
                            # host pre-scales m by s^2; injecting it into the
                            # psum via identity matmul makes psum_m = s^2 * m'
                            if minject:
                                nc.tensor.matmul(
                                    out=psum_m[:, jj * D:(jj + 1) * D],
                                    lhsT=ident[:],
                                    rhs=m_s[:, o0 + jj * D:o0 + (jj + 1) * D],
                                    start=True, stop=False,
                                )
                            for c in range(cpb):
                                k = jj * cpb + c
                                nc.tensor.matmul(
                                    out=psum_m[:, jj * D:(jj + 1) * D],
                                    lhsT=a_s[:, k, :],
                                    rhs=g2_s[:, q0 + k * D:q0 + (k + 1) * D],
                                    start=(not minject and c == 0),
                                    stop=(c == cpb - 1),
                                )
                        for jj in range(jsub):
                            for c in range(cpb):
                                k = jj * cpb + c
                                nc.tensor.matmul(
                                    out=psum_g[:, jj * D:(jj + 1) * D],
                                    lhsT=a_s[:, k, :],
                                    rhs=g_s[:, q0 + k * D:q0 + (k + 1) * D],
                                    start=(c == 0), stop=(c == cpb - 1),
                                )

                        if minject:
                            # m' = psum_m / s^2, on ACT straight from PSUM
                            nc.scalar.activation(
                                m_n[:, o0:o0 + JD], psum_m[:],
                                mybir.ActivationFunctionType.Copy,
                                scale=inv_s2[:])
                        else:
                            nc.vector.affine_then_add(
                                out=m_n[:, o0:o0 + JD], in0=psum_m[:],
                                in1=m_s[:, o0:o0 + JD], scale=inv_s2[:],
                                bias=0.0)

                        s_t = pool.tile([P, JD], f32)
                        if minject:
                            nc.scalar.activation(
                                s_t[:], psum_m[:],
                                mybir.ActivationFunctionType.Sqrt,
                                bias=eps_t[:], scale=inv_s2[:])
                        else:
                            nc.scalar.activation(
                                s_t[:], m_n[:, o0:o0 + JD],
                                mybir.ActivationFunctionType.Sqrt,
                                bias=eps_t[:])
                        r_t = pool.tile([P, JD], f32)
                        nc.vector.reciprocal_approx_fast(out=r_t[:],
                                                         in_=s_t[:])
                        t_t = pool.tile([P, JD], f32)
                        nc.vector.tensor_mul(t_t[:], r_t[:], psum_g[:])
                        nc.gpsimd.tensor_tensor(
                            out=w_n[:, o0:o0 + JD],
                            in0=w_s[:, o0:o0 + JD], in1=t_t[:],
                            op=mybir.AluOpType.add)

                    store.dma_start(out=m_out[:, gc0:gc0 + nbe * JD],
                                    in_=m_n[:])
                    store.dma_start(out=w_out[:, gc0:gc0 + nbe * JD],
                                    in_=w_n[:])

    nc.compile()
    return nc


def get_program(cpb, cap, **opts):
    key = (cpb, cap, tuple(sorted(opts.items())))
    if key not in _program_cache:
        _program_cache[key] = _build_program(cpb, cap, **opts)
    return _program_cache[key]


def prepare_inputs(gradients, weights, moments, indices, learning_rate,
                   valid_count, g_dtype="fp16", t_dtype="fp16"):
    """Host-side routing: shard tables by row range, route gradient rows to
    owning cores, pack into the block/slot layout the device sweep expects."""
    g = np.ascontiguousarray(np.asarray(gradients, dtype=np.float32))
    w = np.asarray(weights, dtype=np.float32)
    m = np.asarray(moments, dtype=np.float32)
    idx = np.asarray(indices).astype(np.int64)
    vc = int(valid_count)
    lr = np.float32(np.asarray(learning_rate).reshape(-1)[0])

    idxv = idx[:vc]
    owner = idxv // VC
    loc = idxv - owner * VC
    j = loc % R
    mstrip = loc // R

    group = owner * R + j
    counts = np.bincount(group, minlength=NCORES * R)
    order = np.argsort(group, kind="stable")
    starts = np.concatenate(([0], np.cumsum(counts)[:-1]))
    rank = np.empty(vc, dtype=np.int64)
    rank[order] = np.arange(vc, dtype=np.int64) - starts[group[order]]

    maxcnt = max(1, int(counts.max()))
    cap = min(P, -(-maxcnt // 16) * 16)  # chunk capacity, multiple of 16
    cpb = -(-maxcnt // cap)              # chunks per block

    colidx = j * cpb + rank // cap
    part = rank % cap

    def np_dt(name):
        if name == "bf16":
            import ml_dtypes
            return ml_dtypes.bfloat16
        return {"fp16": np.float16, "f32": np.float32}[name]

    np_gdt = np_dt(g_dtype)
    np_tdt = np_dt(t_dtype)
    sscale = -lr if lr != 0.0 else 1.0
    g_dev = np.zeros((NCORES, cap, R * cpb, D), dtype=np_gdt)
    g_dev[owner, part, colidx] = (np.float32(sscale) * g[:vc]).astype(np_gdt)
    g_dev = g_dev.reshape(NCORES, cap, R * cpb * D)

    midx_dev = np.zeros((NCORES, cap, R * cpb), dtype=np.float32)
    midx_dev[owner, part, colidx] = mstrip.astype(np.float32)

    w_dev = np.zeros((NCORES, PADV, D), dtype=np_tdt)
    w_dev[:, :VC] = w.reshape(NCORES, VC, D).astype(np_tdt)
    w_dev = w_dev.reshape(NCORES, P, R * D)
    # pre-scaled by s^2 so the device's psum_m = s^2 * m' (identity-matmul
    # inject); a no-op when |lr| == 1
    m_pre = m if sscale * sscale == 1.0 else m * np.float32(sscale * sscale)
    m_dev = np.zeros((NCORES, PADV, D), dtype=np_tdt)
    m_dev[:, :VC] = m_pre.reshape(NCORES, VC, D).astype(np_tdt)
    m_dev = m_dev.reshape(NCORES, P, R * D)

    lr_arr = np.full((1, 1), 1.0 / (sscale * sscale), dtype=np.float32)

    in_maps = [
        {
            "w_in": w_dev[c],
            "m_in": m_dev[c],
            "g_in": g_dev[c],
            "midx": midx_dev[c],
            "lr": lr_arr,
        }
        for c in range(NCORES)
    ]
    return in_maps, cpb, cap


def assemble_outputs(results):
    w_new = np.empty((NCORES * VC, D), dtype=np.float32)
    m_new = np.empty((NCORES * VC, D), dtype=np.float32)
    for c in range(NCORES):
        w_new[c * VC:(c + 1) * VC] = \
            results[c]["w_out"].reshape(PADV, D)[:VC].astype(np.float32)
        m_new[c * VC:(c + 1) * VC] = \
            results[c]["m_out"].reshape(PADV, D)[:VC].astype(np.float32)
    return w_new, m_new


def kernel(gradients, weights, moments, indices, learning_rate, valid_count):
    from concourse.bass_utils import run_bass_kernel_spmd

    lr = float(np.asarray(learning_rate).reshape(-1)[0])
    if lr == 0.0:
        # Degenerate case (never hit with this spec): weights unchanged,
        # moments still accumulate g^2. Compute on host.
        g = np.asarray(gradients, dtype=np.float32).copy()
        g[int(valid_count):] = 0.0
        idx = np.asarray(indices).astype(np.int64)
        m_new = np.asarray(moments, dtype=np.float32).copy()
        np.add.at(m_new, idx, g * g)
        return np.asarray(weights, dtype=np.float32).copy(), m_new

    in_maps, cpb, cap = prepare_inputs(gradients, weights, moments, indices,
                                       learning_rate, valid_count)
    nc = get_program(cpb, cap)
    res = run_bass_kernel_spmd(nc, in_maps, core_ids=list(range(NCORES)))
    return assemble_outputs(res.results)
